# revision 16
# baseline (speedup 1.0000x reference)
"""Trainium2 Bass kernel for local (block-sparse) scaled-dot-product attention.

Contract: kernel(**inputs) takes the FULL inputs of the reference
(query/key_in/value [8, 4096, 512] fp32, Wq/Wk/Wv/Wo [512, 512], biases [512])
and returns the FULL output [8, 4096, 512] fp32.

Sharding: data-parallel over batch; batch element b runs on NeuronCore b.

On-chip layout is feature-major ("transposed"): activations live as [feat, t]
so the contraction dim of every matmul is on partitions. The CPU pre-transposes
the inputs/weights (free) and transposes the output back.

v2 structure (vs the v1 baseline):
- deferred softmax normalization: PV consumes raw exp scores; the output tile
  is multiplied by the broadcast reciprocal denominator instead (halves the
  DVE multiply work and removes the per-head broadcast matmuls).
- window masking via per-partition bias on the Exp activation (two calls per
  head, one per query half-block) instead of rank-2 mask matmuls on PE.
- per-group softmax denominators accumulate into one [8, 512] PSUM tile via
  one-hot lhsT matmuls; a single reciprocal serves all 8 heads.
- bv/bo folded on the host into bo2 = bv @ Wo.T + bo (softmax rows sum to 1).
- PSUM->SBUF copies spread across Act (q, k, v) and DVE (out).
- output stored/DMAed as bf16 and upcast on the host.
- the next group's projection matmuls are interleaved into the current
  group's attention PE stream to hide Act-latency stalls.
"""

import math
from collections import deque

import numpy as np
import ml_dtypes

import concourse.bass as bass
import concourse.tile as tile
from concourse import bacc, mybir
from concourse.bass_utils import run_bass_kernel_spmd

# ---- problem constants (hardcoded; must match the reference) ----
B, T, F = 8, 4096, 512
H, DK, DV = 8, 64, 64
CTX = 64          # block size (cq == ck == 64, nb == 64)
NB = T // CTX     # 64 blocks
SCALE = 1.0 / math.sqrt(DK)
NEGB = -30000.0   # additive mask bias; exp(x + NEGB) == 0.0 in fp32

TG = 8            # t-groups per core
TT = T // TG      # 512 t positions per group
NB8 = TT // CTX   # 8 blocks per group

# bf16 everywhere on the matmul path (fp32 PSUM accumulation).
DT = mybir.dt.bfloat16
NP_DT = ml_dtypes.bfloat16
F32 = mybir.dt.float32
FP8 = mybir.dt.float8e4
NP_FP8 = ml_dtypes.float8_e4m3
WSCALE = 256.0  # fp8 weight prescale (host) undone in the PSUM->SBUF copy

_CACHED = None


def _flat(ap):
    # [p, a, b] -> [p, a*b] view of a contiguous tile
    return ap.rearrange("p a b -> p (a b)")


def _build_consts():
    """Constant tables for masks / reductions / broadcasts.

    bm [128, 3] f32: per-k-partition Exp bias columns:
      0 (A): kill j>=96  (mid blocks, query half 0)
      1 (B): kill j<32   (mid blocks, query half 1)
      2 (C): kill j<32 or j>=96 (edge blocks)
    oneh [128, 1]: all-ones lhsT for per-head denominator column sums
      (M=1 matmul onto partition 0 of a [1, 512] PSUM tile).
    """
    j = np.arange(128)
    bm = np.zeros((128, 3), np.float32)
    bm[j >= 96, 0] = NEGB
    bm[j < 32, 1] = NEGB
    bm[(j < 32) | (j >= 96), 2] = NEGB
    oneh = np.ones((128, 1), np.float32)
    return bm, oneh


def _build_nc(n_iter=1):
    nc = bacc.Bacc(None, target_bir_lowering=False, debug=False)

    xq = nc.dram_tensor("xq", [F, T], FP8, kind="ExternalInput")
    xk = nc.dram_tensor("xk", [F, T], FP8, kind="ExternalInput")
    xv = nc.dram_tensor("xv", [F, T], DT, kind="ExternalInput")
    wq = nc.dram_tensor("wq", [F, F], FP8, kind="ExternalInput")  # Wq.T * WSCALE
    wk = nc.dram_tensor("wk", [F, F], FP8, kind="ExternalInput")  # Wk.T * WSCALE
    wv = nc.dram_tensor("wv", [F, F], DT, kind="ExternalInput")  # Wv.T
    wo = nc.dram_tensor("wo", [F, F], DT, kind="ExternalInput")  # Wo.T
    bq = nc.dram_tensor("bq", [F], F32, kind="ExternalInput")
    bk = nc.dram_tensor("bk", [F], F32, kind="ExternalInput")
    bo2 = nc.dram_tensor("bo2", [F], F32, kind="ExternalInput")  # bv@Wo.T + bo
    bm = nc.dram_tensor("bm", [128, 3], F32, kind="ExternalInput")
    oneh = nc.dram_tensor("oneh", [128, 1], DT, kind="ExternalInput")
    outd = nc.dram_tensor("out", [F, T], DT, kind="ExternalOutput")

    Exp = mybir.ActivationFunctionType.Exp
    Identity = mybir.ActivationFunctionType.Identity

    with tile.TileContext(nc) as tc:
        with (
            tc.tile_pool(name="singles", bufs=1) as singles,
            tc.tile_pool(name="xin", bufs=2) as xin,
            tc.tile_pool(name="proj_out", bufs=2) as pqk,
            tc.tile_pool(name="vpool", bufs=2) as vpool,
            tc.tile_pool(name="epool", bufs=10) as epool,
            tc.tile_pool(name="rpool", bufs=10) as rpool,
            tc.tile_pool(name="ypool", bufs=2) as ypool,
            tc.tile_pool(name="opool", bufs=2) as opool,
            tc.tile_pool(name="ps_proj", bufs=2, space="PSUM") as ps_proj,
            tc.tile_pool(name="ps_s", bufs=2, space="PSUM") as ps_s,
            tc.tile_pool(name="ps_sum", bufs=2, space="PSUM") as ps_sum,
            tc.tile_pool(name="ps_o", bufs=2, space="PSUM") as ps_o,
        ):
            # ---- static tiles ----
            wq_t = singles.tile([128, 4, F], FP8, tag="wq")
            wk_t = singles.tile([128, 4, F], FP8, tag="wk")
            wv_t = singles.tile([128, 4, F], DT, tag="wv")
            wo_t = singles.tile([128, 4, F], DT, tag="wo")
            bq_t = singles.tile([128, 4], F32, tag="bq")
            bk_t = singles.tile([128, 4], F32, tag="bk")
            bo2_t = singles.tile([128, 4], F32, tag="bo2")
            bm_t = singles.tile([128, 3], F32, tag="bm")
            oneh_t = singles.tile([128, 1], DT, tag="oneh")

            def emit_singles_front():
                nc.sync.dma_start(
                    out=wq_t, in_=wq.rearrange("(c p) o -> p c o", p=128)
                )
                nc.sync.dma_start(out=bq_t, in_=bq.rearrange("(c p) -> p c", p=128))
                nc.sync.dma_start(out=bm_t, in_=bm[:, :])

            def emit_singles_rest():
                for wt, wd in ((wk_t, wk), (wv_t, wv), (wo_t, wo)):
                    nc.sync.dma_start(
                        out=wt, in_=wd.rearrange("(c p) o -> p c o", p=128)
                    )
                for bt, bd in ((bk_t, bk), (bo2_t, bo2)):
                    nc.sync.dma_start(
                        out=bt, in_=bd.rearrange("(c p) -> p c", p=128)
                    )
                nc.sync.dma_start(out=oneh_t, in_=oneh[:, :])

            xq_r = xq.rearrange("(c p) t -> p c t", p=128)
            xk_r = xk.rearrange("(c p) t -> p c t", p=128)
            xv_r = xv.rearrange("(c p) t -> p c t", p=128)
            out_r = outd.rearrange("(c p) t -> p c t", p=128)

            def emit_loads(tg):
                t0 = tg * TT
                st = {}
                st["xq_s"] = xin.tile([128, 4, TT], FP8, tag="xq", name="xq_s")
                nc.sync.dma_start(out=st["xq_s"], in_=xq_r[:, :, t0 : t0 + TT])
                lo, hi = t0 - 32, t0 + TT + 32
                clo, chi = max(lo, 0), min(hi, T)
                st["xk_s"] = xin.tile([128, 4, TT + 64], FP8, tag="xk", name="xk_s")
                st["xv_s"] = xin.tile([128, 4, TT + 64], DT, tag="xv", name="xv_s")
                for xs, xr in ((st["xk_s"], xk_r), (st["xv_s"], xv_r)):
                    nc.sync.dma_start(
                        out=xs[:, :, clo - lo : chi - lo], in_=xr[:, :, clo:chi]
                    )
                    if clo > lo:
                        nc.vector.memset(xs[:, :, 0 : clo - lo], 0.0)
                    if chi < hi:
                        nc.vector.memset(xs[:, :, TT + 64 - (hi - chi) :], 0.0)
                st["qT"] = pqk.tile([128, 4, TT], DT, tag="qT", name="qT")
                st["kT"] = pqk.tile([128, 4, TT + 64], DT, tag="kT", name="kT")
                st["v0"] = vpool.tile([128, 5, F], DT, tag="v0", name="v0")
                st["v0s"] = vpool.tile([128, 4, F], DT, tag="v0s", name="v0s")
                return st

            def make_proj_chunks(st):
                xq_s, xk_s, xv_s = st["xq_s"], st["xk_s"], st["xv_s"]
                qT, kT, v0, v0s = st["qT"], st["kT"], st["v0"], st["v0s"]

                def q_chunk(oc):
                    def f():
                        ps = ps_proj.tile([128, 512], F32, tag="proj")
                        for fc2 in range(2):
                            nc.tensor.matmul(
                                ps,
                                lhsT=wq_t[:, 2 * fc2 : 2 * fc2 + 2,
                                          oc * 128 : (oc + 1) * 128],
                                rhs=xq_s[:, 2 * fc2 : 2 * fc2 + 2, :],
                                start=(fc2 == 0),
                                stop=(fc2 == 1),
                                perf_mode=mybir.MatmulPerfMode.DoubleRow,
                            )
                        nc.scalar.activation(
                            out=qT[:, oc, :], in_=ps, func=Identity,
                            bias=bq_t[:, oc : oc + 1], scale=1.0 / WSCALE,
                        )
                    return f

                def k_chunk(oc):
                    def f():
                        ps = ps_proj.tile([128, 512], F32, tag="proj")
                        for fc2 in range(2):
                            nc.tensor.matmul(
                                ps,
                                lhsT=wk_t[:, 2 * fc2 : 2 * fc2 + 2,
                                          oc * 128 : (oc + 1) * 128],
                                rhs=xk_s[:, 2 * fc2 : 2 * fc2 + 2, 0:512],
                                start=(fc2 == 0),
                                stop=(fc2 == 1),
                                perf_mode=mybir.MatmulPerfMode.DoubleRow,
                            )
                        nc.vector.tensor_scalar(
                            out=kT[:, oc, 0:512], in0=ps,
                            scalar1=1.0 / WSCALE, scalar2=bk_t[:, oc : oc + 1],
                            op0=mybir.AluOpType.mult, op1=mybir.AluOpType.add,
                        )
                        ps2 = ps_proj.tile([128, 64], F32, tag="proj")
                        for fc2 in range(2):
                            nc.tensor.matmul(
                                ps2,
                                lhsT=wk_t[:, 2 * fc2 : 2 * fc2 + 2,
                                          oc * 128 : (oc + 1) * 128],
                                rhs=xk_s[:, 2 * fc2 : 2 * fc2 + 2, 512:576],
                                start=(fc2 == 0),
                                stop=(fc2 == 1),
                                perf_mode=mybir.MatmulPerfMode.DoubleRow,
                            )
                        nc.vector.tensor_scalar(
                            out=kT[:, oc, 512:576], in0=ps2,
                            scalar1=1.0 / WSCALE, scalar2=bk_t[:, oc : oc + 1],
                            op0=mybir.AluOpType.mult, op1=mybir.AluOpType.add,
                        )
                    return f

                def v_chunk(tc5):
                    def f():
                        m = 128 if tc5 < 4 else 64
                        ps = ps_proj.tile([128, 512], F32, tag="proj")
                        for fc in range(4):
                            nc.tensor.matmul(
                                ps[0:m, :],
                                lhsT=xv_s[:, fc, 128 * tc5 : 128 * tc5 + m],
                                rhs=wv_t[:, fc, :],
                                start=(fc == 0),
                                stop=(fc == 3),
                            )
                        nc.scalar.copy(out=v0[0:m, tc5, :], in_=ps[0:m, :])
                    return f

                def v_shift():
                    # shifted copy: v0s covers [t0+32, t0+544), chunk c = rows
                    # [64..128) of v0 chunk c plus rows [0..64) of chunk c+1.
                    nc.sync.dma_start(out=v0s[0:64, :, :], in_=v0[64:128, 0:4, :])
                    nc.sync.dma_start(out=v0s[64:128, :, :], in_=v0[0:64, 1:5, :])

                return [
                    q_chunk(0), q_chunk(1), q_chunk(2), q_chunk(3),
                    k_chunk(0), k_chunk(1), k_chunk(2), k_chunk(3),
                    v_chunk(0), v_chunk(1), v_chunk(2), v_chunk(3), v_chunk(4),
                    v_shift,
                ]

            def emit_exp(tg, eT, sT):
                bA = bm_t[:, 0:1]
                bB = bm_t[:, 1:2]
                bC = bm_t[:, 2:3]
                # query half 0 (q in [0, 32)): valid keys are j in [0, 96)
                if tg == 0:
                    # global block 0 has no left neighbor: also kill j < 32
                    nc.scalar.activation(
                        out=eT[:, 0:1, 0:32], in_=sT[:, 0:1, 0:32],
                        func=Exp, bias=bC, scale=SCALE,
                    )
                    nc.scalar.activation(
                        out=eT[:, 1:NB8, 0:32], in_=sT[:, 1:NB8, 0:32],
                        func=Exp, bias=bA, scale=SCALE,
                    )
                else:
                    nc.scalar.activation(
                        out=eT[:, :, 0:32], in_=sT[:, :, 0:32],
                        func=Exp, bias=bA, scale=SCALE,
                    )
                # query half 1 (q in [32, 64)): valid keys are j in [32, 128)
                if tg == TG - 1:
                    nc.scalar.activation(
                        out=eT[:, 0 : NB8 - 1, 32:64], in_=sT[:, 0 : NB8 - 1, 32:64],
                        func=Exp, bias=bB, scale=SCALE,
                    )
                    # global block 63 has no right neighbor: also kill j >= 96
                    nc.scalar.activation(
                        out=eT[:, NB8 - 1 : NB8, 32:64],
                        in_=sT[:, NB8 - 1 : NB8, 32:64],
                        func=Exp, bias=bC, scale=SCALE,
                    )
                else:
                    nc.scalar.activation(
                        out=eT[:, :, 32:64], in_=sT[:, :, 32:64],
                        func=Exp, bias=bB, scale=SCALE,
                    )

            def emit_attn(tg, st, fillers):
                t0_unused = tg  # noqa: F841
                qT, kT, v0, v0s = st["qT"], st["kT"], st["v0"], st["v0s"]

                def fill(n):
                    for _ in range(n):
                        if fillers:
                            fillers.popleft()()

                eTs = []
                eNs = []
                for hp in range(4):
                    sT0 = ps_s.tile([128, NB8, 64], F32, tag="sT")
                    sT1 = ps_s.tile([128, NB8, 64], F32, tag="sT")
                    sTs = (sT0, sT1)
                    for n8 in range(NB8):
                        for hl in range(2):
                            pb = hl * 64
                            nc.tensor.matmul(
                                sTs[hl][:, n8, :],
                                lhsT=kT[pb : pb + 64, hp, 64 * n8 : 64 * n8 + 128],
                                rhs=qT[pb : pb + 64, hp, 64 * n8 : 64 * n8 + 64],
                                start=True,
                                stop=True,
                            )
                    for hl in range(2):
                        eT = epool.tile([128, NB8, 64], DT, tag="eT")
                        emit_exp(tg, eT, sTs[hl])
                        eTs.append(eT)
                    fill(2)
                    # per-head [1, 512] sums at partition 0: the only layout
                    # the gpsimd PartitionBroadcast source supports.
                    for hl in range(2):
                        sums_h = ps_sum.tile(
                            [1, 512], F32, tag="sums", name="sums_h"
                        )
                        nc.tensor.matmul(
                            sums_h,
                            lhsT=oneh_t[:, :],
                            rhs=_flat(eTs[2 * hp + hl]),
                            start=True,
                            stop=True,
                        )
                        rs_h = rpool.tile([1, 512], DT, tag="rs", name="rs_h")
                        with nc.allow_low_precision(reason="bf16 denoms"):
                            nc.vector.reciprocal(out=rs_h, in_=sums_h)
                        bc = rpool.tile([128, 512], DT, tag="bc", name="bc")
                        nc.gpsimd.partition_broadcast(bc, rs_h[0:1, :])
                        # all-SBUF bf16 multiply -> DVE 4x fast path
                        eN = epool.tile([128, NB8, 64], DT, tag="eN", name="eN")
                        nc.vector.tensor_mul(_flat(eN), _flat(eTs[2 * hp + hl]), bc)
                        eNs.append(eN)
                    fill(1)

                yT = ypool.tile([128, 4, TT], DT, tag="yT")
                for hp in range(4):
                    oT = ps_o.tile([128, 512], F32, tag="oT")
                    for hl in range(2):
                        h = 2 * hp + hl
                        pb = hl * 64
                        for n8 in range(NB8):
                            if n8 % 2 == 0:
                                lhsT = v0[:, n8 // 2, 64 * h : 64 * h + 64]
                            else:
                                lhsT = v0s[:, (n8 - 1) // 2, 64 * h : 64 * h + 64]
                            nc.tensor.matmul(
                                oT[pb : pb + 64, 64 * n8 : 64 * n8 + 64],
                                lhsT=lhsT,
                                rhs=eNs[h][:, n8, :],
                                start=True,
                                stop=True,
                                tile_position=(0, pb),
                            )
                    nc.scalar.copy(out=yT[:, hp, :], in_=oT)
                    fill(2)

                outsb = opool.tile([128, 4, TT], DT, tag="outsb")
                for oc in range(4):
                    ps = ps_proj.tile([128, 512], F32, tag="proj")
                    for fc in range(4):
                        nc.tensor.matmul(
                            ps,
                            lhsT=wo_t[:, fc, oc * 128 : (oc + 1) * 128],
                            rhs=yT[:, fc, :],
                            start=(fc == 0),
                            stop=(fc == 3),
                        )
                    nc.vector.tensor_scalar_add(
                        outsb[:, oc, :], ps, bo2_t[:, oc : oc + 1]
                    )
                    fill(1)
                nc.sync.dma_start(
                    out=out_r[:, :, tg * TT : tg * TT + TT], in_=outsb
                )
                fill(len(fillers))

            def emit_all():
                emit_singles_front()
                st = emit_loads(0)
                emit_singles_rest()
                for c in make_proj_chunks(st):
                    c()
                for tg in range(TG):
                    if tg + 1 < TG:
                        st_next = emit_loads(tg + 1)
                        fillers = deque(make_proj_chunks(st_next))
                    else:
                        st_next = None
                        fillers = deque()
                    emit_attn(tg, st, fillers)
                    st = st_next

            if n_iter == 1:
                emit_all()
            else:
                with tc.For_i(0, n_iter, 1):
                    emit_all()

    nc.finalize()
    return nc


def _get_nc(n_iter=1):
    global _CACHED
    if _CACHED is None:
        _CACHED = {}
    if n_iter not in _CACHED:
        _CACHED[n_iter] = _build_nc(n_iter)
    return _CACHED[n_iter]


def _prep_in_maps(query, key_in, value, Wq, bq, Wk, bk, Wv, bv, Wo, bo):
    bm, oneh = _build_consts()
    bo2 = np.asarray(bv, np.float32) @ np.asarray(Wo, np.float32).T + np.asarray(
        bo, np.float32
    )
    shared = {
        "wq": np.ascontiguousarray(np.asarray(Wq, np.float32).T * WSCALE).astype(
            NP_FP8
        ),
        "wk": np.ascontiguousarray(np.asarray(Wk, np.float32).T * WSCALE).astype(
            NP_FP8
        ),
        "wv": np.ascontiguousarray(Wv.T).astype(NP_DT),
        "wo": np.ascontiguousarray(Wo.T).astype(NP_DT),
        "bq": np.asarray(bq, np.float32),
        "bk": np.asarray(bk, np.float32),
        "bo2": bo2,
        "bm": bm,
        "oneh": oneh.astype(NP_DT),
    }
    from concurrent.futures import ThreadPoolExecutor

    def _tp(a):
        return np.ascontiguousarray(np.asarray(a, np.float32).T.astype(NP_DT))

    def _tp8(a):
        return np.ascontiguousarray(np.asarray(a, np.float32).T.astype(NP_FP8))

    with ThreadPoolExecutor(12) as ex:
        xqs = list(ex.map(_tp8, [query[b] for b in range(B)]))
        xks = list(ex.map(_tp8, [key_in[b] for b in range(B)]))
        xvs = list(ex.map(_tp, [value[b] for b in range(B)]))
    in_maps = []
    for b in range(B):
        in_maps.append({"xq": xqs[b], "xk": xks[b], "xv": xvs[b], **shared})
    return in_maps


def run(trace=False, **inputs):
    nc = _get_nc()
    in_maps = _prep_in_maps(**inputs)
    res = run_bass_kernel_spmd(
        nc, in_maps, core_ids=list(range(B)), trace=trace
    )
    out = np.stack(
        [
            np.asarray(res.results[b]["out"]).astype(np.float32).T
            for b in range(B)
        ]
    )
    return out, res


def kernel(**inputs):
    out, _ = run(trace=False, **inputs)
    return out
